# revision 2
# baseline (speedup 1.0000x reference)
"""KiloNeRF Trainium2 kernel: 4096 tiny MLPs, 512 points each, 8 NeuronCores.

Sharding: expert-parallel along the network axis (512 nets/core). Host-side
numpy packs per-core inputs into feature-major, PE-friendly layouts; the
device kernel is a stream of full-array block-diagonal matmuls (4 nets per
128-partition tile), bf16 inputs with f32 PSUM accumulation.

v2: the feature layer is linear (no relu) and not an output, so it is folded
into the direction layer on the host: Wfd = Wd_f @ Wf, bfd = Wd_f @ bf + bd.
This removes one matmul and one PSUM eviction per group (7 matmuls, 4
evictions instead of 8/5).
"""

import sys

sys.path.insert(0, "/opt/trn_rl_repo")

import numpy as np
import ml_dtypes

N_NET = 4096
P = 512
PC = 63
DC = 27
H = 32
NCORES = 8
NPC = N_NET // NCORES  # nets per core = 512
NPG = 4  # nets per group (one 128-partition tile)
G = NPC // NPG  # groups per core = 128

BF16 = ml_dtypes.bfloat16

_nc_cache = {}


def _build_nc():
    import concourse.mybir as mybir
    import concourse.tile as tile
    from concourse import bacc

    nc = bacc.Bacc("TRN2")
    dt = mybir.dt
    AF = mybir.ActivationFunctionType
    ALU = mybir.AluOpType

    with tile.TileContext(nc) as tc:
        posT_d = nc.dram_tensor("posT", [G, 2, 128, P], dt.bfloat16, kind="ExternalInput")
        dirT_d = nc.dram_tensor("dirT", [G, 128, P], dt.bfloat16, kind="ExternalInput")
        w0_d = nc.dram_tensor("w0", [G, 128, 128], dt.bfloat16, kind="ExternalInput")
        wdiag_d = nc.dram_tensor("wdiag", [G, 4, 32, 5, 32], dt.bfloat16, kind="ExternalInput")
        bias_d = nc.dram_tensor("bias", [G, 128, 8], dt.float32, kind="ExternalInput")
        out_d = nc.dram_tensor("out", [G, 16, P], dt.float32, kind="ExternalOutput")

        NCV = 3  # weight-canvas ring depth
        with (
            tc.tile_pool(name="cv", bufs=1) as cvp,
            tc.tile_pool(name="io", bufs=3) as io,
            tc.tile_pool(name="act", bufs=3) as actp,
            tc.tile_pool(name="ps0", bufs=2, space="PSUM") as ps0,
            tc.tile_pool(name="ps1", bufs=2, space="PSUM") as ps1,
            tc.tile_pool(name="psd", bufs=2, space="PSUM") as psd,
            tc.tile_pool(name="pso", bufs=2, space="PSUM") as pso,
        ):
            # Persistent zero canvases for the 5 diagonal weight matrices
            # (w1, wfd, wdd, wr, wa). Only the 32x32 diagonal blocks are ever
            # DMA-written; off-diagonal zeros from this one-time memset persist.
            canvases = []
            for i in range(NCV):
                cv = cvp.tile([128, 5 * 128], dt.bfloat16, tag=f"cv{i}")
                nc.vector.memset(cv[:], 0.0)
                canvases.append(cv)

            for g in range(G):
                cv = canvases[g % NCV]
                # --- loads ---
                pos0 = io.tile([128, P], dt.bfloat16, tag="pos0")
                pos1 = io.tile([128, P], dt.bfloat16, tag="pos1")
                dirt = io.tile([128, P], dt.bfloat16, tag="dirt")
                w0 = io.tile([128, 128], dt.bfloat16, tag="w0")
                bia = io.tile([128, 8], dt.float32, tag="bias")
                nc.sync.dma_start(out=pos0[:], in_=posT_d[g, 0])
                nc.sync.dma_start(out=pos1[:], in_=posT_d[g, 1])
                nc.sync.dma_start(out=dirt[:], in_=dirT_d[g])
                nc.sync.dma_start(out=w0[:], in_=w0_d[g])
                nc.sync.dma_start(out=bia[:], in_=bias_d[g])
                for j in range(4):
                    # diag blocks for strip j of all 5 matrices in one DMA:
                    # dst [32 parts, 5 mats (col stride 128), 32 cols]
                    nc.sync.dma_start(
                        out=cv[32 * j : 32 * j + 32].rearrange(
                            "p (m c) -> p m c", m=5
                        )[:, :, 32 * j : 32 * j + 32],
                        in_=wdiag_d[g, j],
                    )

                # --- L0: h1 = relu(pos @ W0^T + b0) ---
                p_l0 = ps0.tile([128, P], dt.float32, tag="l0")
                nc.tensor.matmul(p_l0[0:64], lhsT=w0[:, 0:64], rhs=pos0[:], start=True, stop=True)
                nc.tensor.matmul(p_l0[64:128], lhsT=w0[:, 64:128], rhs=pos1[:], start=True, stop=True)
                h1 = actp.tile([128, P], dt.bfloat16, tag="h1")
                nc.scalar.activation(h1[:], p_l0[:], AF.Relu, bias=bia[:, 0:1], scale=1.0)

                # --- L1: h2 = relu(h1 @ W1^T + b1) ---
                p_l1 = ps1.tile([128, P], dt.float32, tag="l1")
                nc.tensor.matmul(p_l1[:], lhsT=cv[:, 0:128], rhs=h1[:], start=True, stop=True)
                h2 = actp.tile([128, P], dt.bfloat16, tag="h2")
                nc.vector.tensor_scalar(h2[:], p_l1[:], bia[:, 1:2], 0.0, op0=ALU.add, op1=ALU.max)

                # --- Ld: h3 = relu(h2 @ Wfd^T + dir @ Wdd^T + bfd)  (feat folded) ---
                p_ld = psd.tile([128, P], dt.float32, tag="ld")
                nc.tensor.matmul(p_ld[:], lhsT=cv[:, 128:256], rhs=h2[:], start=True, stop=False)
                nc.tensor.matmul(p_ld[:], lhsT=cv[:, 256:384], rhs=dirt[:], start=False, stop=True)
                h3 = actp.tile([128, P], dt.bfloat16, tag="h3")
                nc.vector.tensor_scalar(h3[:], p_ld[:], bia[:, 2:3], 0.0, op0=ALU.add, op1=ALU.max)

                # --- Lout: [rgb | alpha] = h3 @ Wr^T , h2 @ Wa^T (+ bias) ---
                p_lo = pso.tile([128, P], dt.float32, tag="lo")
                nc.tensor.matmul(p_lo[:], lhsT=cv[:, 384:512], rhs=h3[:], start=True, stop=False)
                nc.tensor.matmul(p_lo[:], lhsT=cv[:, 512:640], rhs=h2[:], start=False, stop=True)
                ob = actp.tile([128, P], dt.float32, tag="ob")
                nc.scalar.activation(ob[:], p_lo[:], AF.Identity, bias=bia[:, 3:4], scale=1.0)
                for j in range(4):
                    nc.sync.dma_start(
                        out=out_d[g, 4 * j : 4 * j + 4], in_=ob[32 * j : 32 * j + 4]
                    )

    nc.compile()
    return nc


def _pack_core(c, x, W0, b0, W1, b1, Wa, ba, Wf, bf, Wd, bd, Wr, br):
    lo, hi = c * NPC, (c + 1) * NPC
    xT = np.ascontiguousarray(
        x[lo:hi].transpose(0, 2, 1)
    )  # [512, 90, 512] f32 feature-major

    posT = np.zeros((G, 2, 128, P), dtype=BF16)
    pt = xT[:, :PC, :].astype(BF16).reshape(G, 4, PC, P)
    posT[:, 0, 0:PC] = pt[:, 0]
    posT[:, 0, 64 : 64 + PC] = pt[:, 1]
    posT[:, 1, 0:PC] = pt[:, 2]
    posT[:, 1, 64 : 64 + PC] = pt[:, 3]

    dirT = np.zeros((G, 128, P), dtype=BF16)
    dd = xT[:, PC:, :].astype(BF16).reshape(G, 4, DC, P)
    for j in range(4):
        dirT[:, 32 * j : 32 * j + DC] = dd[:, j]

    # weights, feature-major lhsT blocks ([in,out] = W^T)
    w0T = W0[lo:hi].transpose(0, 2, 1).astype(BF16).reshape(G, 4, PC, H)
    w0p = np.zeros((G, 128, 128), dtype=BF16)
    for j in range(4):
        r = 64 * (j % 2)
        w0p[:, r : r + PC, 32 * j : 32 * j + 32] = w0T[:, j]

    # fold the (linear, non-output) feature layer into the direction layer:
    # Wfd = Wd_f @ Wf, bfd = Wd_f @ bf + bd
    Wd_f = Wd[lo:hi, :, :H]  # [n, 32(out), 32(feat-in)]
    Wfd = np.einsum("nof,fih->noh", Wd_f, np.eye(H, dtype=np.float32)) * 0  # placeholder shape
    Wfd = np.matmul(Wd_f, Wf[lo:hi])  # [n, 32(out), 32(h2-in)]
    bfd = np.einsum("nof,nf->no", Wd_f, bf[lo:hi]) + bd[lo:hi]

    wdiag = np.zeros((G, 4, 32, 5, 32), dtype=BF16)
    w1T = W1[lo:hi].transpose(0, 2, 1).astype(BF16).reshape(G, 4, H, H)
    wfdT = Wfd.transpose(0, 2, 1).astype(BF16).reshape(G, 4, H, H)
    wddT = Wd[lo:hi, :, H:].transpose(0, 2, 1).astype(BF16).reshape(G, 4, DC, H)
    wrT = Wr[lo:hi].transpose(0, 2, 1).astype(BF16).reshape(G, 4, H, 3)
    waT = Wa[lo:hi].transpose(0, 2, 1).astype(BF16).reshape(G, 4, H, 1)
    wdiag[:, :, :, 0, :] = w1T
    wdiag[:, :, :, 1, :] = wfdT
    wdiag[:, :, :DC, 2, :] = wddT
    wdiag[:, :, :, 3, 0:3] = wrT
    wdiag[:, :, :, 4, 3:4] = waT

    bias = np.zeros((G, 128, 8), dtype=np.float32)
    bias[:, :, 0] = b0[lo:hi].reshape(G, 128)
    bias[:, :, 1] = b1[lo:hi].reshape(G, 128)
    bias[:, :, 2] = bfd.reshape(G, 128)
    bo = np.zeros((G, 4, 32), dtype=np.float32)
    bo[:, :, 0:3] = br[lo:hi].reshape(G, 4, 3)
    bo[:, :, 3] = ba[lo:hi].reshape(G, 4)
    bias[:, :, 3] = bo.reshape(G, 128)

    return {
        "posT": posT,
        "dirT": dirT,
        "w0": w0p,
        "wdiag": wdiag,
        "bias": bias,
    }


def kernel(**inputs):
    from concourse.bass_utils import run_bass_kernel_spmd

    if "nc" not in _nc_cache:
        _nc_cache["nc"] = _build_nc()
    nc = _nc_cache["nc"]

    from concurrent.futures import ThreadPoolExecutor

    with ThreadPoolExecutor(max_workers=8) as ex:
        in_maps = list(ex.map(lambda c: _pack_core(c, **inputs), range(NCORES)))

    res = run_bass_kernel_spmd(nc, in_maps, core_ids=list(range(NCORES)))

    out = np.empty((N_NET, P, 4), dtype=np.float32)
    for c in range(NCORES):
        o = res.results[c]["out"]  # [G, 16, P]
        out[c * NPC : (c + 1) * NPC] = o.reshape(G * NPG, 4, P).transpose(0, 2, 1)
    return out


# revision 4
# speedup vs baseline: 2.4092x; 2.4092x over previous
"""KiloNeRF Trainium2 kernel: 4096 tiny MLPs, 512 points each, 8 NeuronCores.

Sharding: expert-parallel along the network axis (512 nets/core). Host-side
numpy packs per-core inputs into feature-major, PE-friendly layouts; the
device kernel is a stream of full-array block-diagonal matmuls (4 nets per
128-partition tile), bf16 inputs with f32 PSUM accumulation.

v3 changes vs baseline:
- Feature layer (linear, not an output) folded into the direction layer on
  host: Wfd = Wd_f @ Wf, bfd = Wd_f @ bf + bd  -> 7 matmuls / 4 evictions
  per group instead of 8 / 5.
- DMA issue was the bottleneck (each dma_start = ~617 ns serialized on the
  Sync sequencer; 13/group = 1.03 ms). All transfers are now batched over
  supergroups of 8 groups through big double-buffered SBUF tiles:
  10 dma_starts per 8 groups (~1.25/group).
- All per-group biases preloaded in one DMA at kernel start.
"""

import sys

sys.path.insert(0, "/opt/trn_rl_repo")

import numpy as np
import ml_dtypes

N_NET = 4096
P = 512
PC = 63
DC = 27
H = 32
NCORES = 8
NPC = N_NET // NCORES  # nets per core = 512
NPG = 4  # nets per group (one 128-partition tile)
G = NPC // NPG  # groups per core = 128
R = 8  # groups per supergroup
SG = G // R  # supergroups = 16

BF16 = ml_dtypes.bfloat16

_nc_cache = {}


def _build_nc():
    import concourse.mybir as mybir
    import concourse.tile as tile
    from concourse import bacc

    nc = bacc.Bacc("TRN2")
    dt = mybir.dt
    AF = mybir.ActivationFunctionType
    ALU = mybir.AluOpType

    XW = 3 * P  # x cols per group: pos0 | pos1 | dir
    CW = 5 * 128  # canvas cols per group: w1 | wfd | wdd | wr | wa

    with tile.TileContext(nc) as tc:
        x_d = nc.dram_tensor("xin", [SG, 128, R * XW], dt.bfloat16, kind="ExternalInput")
        w0_d = nc.dram_tensor("w0", [SG, 128, R * 128], dt.bfloat16, kind="ExternalInput")
        wdiag_d = nc.dram_tensor("wdiag", [SG, 4, 32, R, 5, 32], dt.bfloat16, kind="ExternalInput")
        bias_d = nc.dram_tensor("bias", [128, G * 4], dt.float32, kind="ExternalInput")
        out_d = nc.dram_tensor("out", [SG, 4, 4, R, P], dt.float32, kind="ExternalOutput")

        with (
            tc.tile_pool(name="big", bufs=1) as bigp,
            tc.tile_pool(name="act", bufs=3) as actp,
            tc.tile_pool(name="ps0", bufs=2, space="PSUM") as ps0,
            tc.tile_pool(name="ps1", bufs=2, space="PSUM") as ps1,
            tc.tile_pool(name="psd", bufs=2, space="PSUM") as psd,
            tc.tile_pool(name="pso", bufs=2, space="PSUM") as pso,
        ):
            biasAll = bigp.tile([128, G * 4], dt.float32, tag="biasAll")
            nc.sync.dma_start(out=biasAll[:], in_=bias_d[:])

            xbig = [bigp.tile([128, R * XW], dt.bfloat16, tag=f"x{i}", name=f"x{i}") for i in range(2)]
            w0big = [bigp.tile([128, R * 128], dt.bfloat16, tag=f"w0{i}", name=f"w0{i}") for i in range(2)]
            cvbig = [bigp.tile([128, R * CW], dt.bfloat16, tag=f"cv{i}", name=f"cv{i}") for i in range(2)]
            obbig = [bigp.tile([128, R * P], dt.float32, tag=f"ob{i}", name=f"ob{i}") for i in range(2)]
            # one-time zero canvases; per-supergroup DMAs only rewrite the
            # 32x32 diagonal blocks, the off-diagonal zeros persist.
            nc.vector.memset(cvbig[0][:], 0.0)
            nc.vector.memset(cvbig[1][:], 0.0)

            for s in range(SG):
                b = s % 2
                xb, w0b, cvb, obb = xbig[b], w0big[b], cvbig[b], obbig[b]
                nc.sync.dma_start(out=xb[:], in_=x_d[s])
                nc.sync.dma_start(out=w0b[:], in_=w0_d[s])
                for j in range(4):
                    # diag blocks, strip j, for all 5 mats x R groups in one
                    # DMA: dst [32 parts, R rings, 5 mats, 32 cols]
                    nc.sync.dma_start(
                        out=cvb[32 * j : 32 * j + 32].rearrange(
                            "p (r m c) -> p r m c", r=R, m=5
                        )[:, :, :, 32 * j : 32 * j + 32],
                        in_=wdiag_d[s, j],
                    )

                for r in range(R):
                    g = R * s + r
                    pos0 = xb[:, r * XW : r * XW + P]
                    pos1 = xb[:, r * XW + P : r * XW + 2 * P]
                    dirt = xb[:, r * XW + 2 * P : r * XW + 3 * P]
                    w0 = w0b[:, r * 128 : (r + 1) * 128]
                    cv = cvb[:, r * CW : (r + 1) * CW]

                    def bia(i):
                        return biasAll[:, 4 * g + i : 4 * g + i + 1]

                    # --- L0: h1 = relu(pos @ W0^T + b0) ---
                    p_l0 = ps0.tile([128, P], dt.float32, tag="l0")
                    nc.tensor.matmul(p_l0[0:64], lhsT=w0[:, 0:64], rhs=pos0, start=True, stop=True)
                    nc.tensor.matmul(p_l0[64:128], lhsT=w0[:, 64:128], rhs=pos1, start=True, stop=True)
                    h1 = actp.tile([128, P], dt.bfloat16, tag="h1")
                    nc.scalar.activation(h1[:], p_l0[:], AF.Relu, bias=bia(0), scale=1.0)

                    # --- L1: h2 = relu(h1 @ W1^T + b1) ---
                    p_l1 = ps1.tile([128, P], dt.float32, tag="l1")
                    nc.tensor.matmul(p_l1[:], lhsT=cv[:, 0:128], rhs=h1[:], start=True, stop=True)
                    h2 = actp.tile([128, P], dt.bfloat16, tag="h2")
                    nc.vector.tensor_scalar(h2[:], p_l1[:], bia(1), 0.0, op0=ALU.add, op1=ALU.max)

                    # --- Ld: h3 = relu(h2 @ Wfd^T + dir @ Wdd^T + bfd) ---
                    p_ld = psd.tile([128, P], dt.float32, tag="ld")
                    nc.tensor.matmul(p_ld[:], lhsT=cv[:, 128:256], rhs=h2[:], start=True, stop=False)
                    nc.tensor.matmul(p_ld[:], lhsT=cv[:, 256:384], rhs=dirt, start=False, stop=True)
                    h3 = actp.tile([128, P], dt.bfloat16, tag="h3")
                    nc.vector.tensor_scalar(h3[:], p_ld[:], bia(2), 0.0, op0=ALU.add, op1=ALU.max)

                    # --- Lout: [rgb | alpha] = h3 @ Wr^T , h2 @ Wa^T (+ bias) ---
                    p_lo = pso.tile([128, P], dt.float32, tag="lo")
                    nc.tensor.matmul(p_lo[:], lhsT=cv[:, 384:512], rhs=h3[:], start=True, stop=False)
                    nc.tensor.matmul(p_lo[:], lhsT=cv[:, 512:640], rhs=h2[:], start=False, stop=True)
                    nc.scalar.activation(
                        obb[:, r * P : (r + 1) * P], p_lo[:], AF.Identity, bias=bia(3), scale=1.0
                    )

                for j in range(4):
                    nc.sync.dma_start(
                        out=out_d[s, j],
                        in_=obb[32 * j : 32 * j + 4].rearrange("p (r c) -> p r c", r=R),
                    )

    nc.compile()
    return nc


def _pack_core(c, x, W0, b0, W1, b1, Wa, ba, Wf, bf, Wd, bd, Wr, br):
    lo, hi = c * NPC, (c + 1) * NPC
    XW = 3 * P
    xT = np.ascontiguousarray(
        x[lo:hi].transpose(0, 2, 1)
    )  # [512, 90, 512] f32 feature-major

    # x big tiles: [SG, 128, R*(pos0|pos1|dir)]
    xbig = np.zeros((G, 128, 3, P), dtype=BF16)
    pt = xT[:, :PC, :].astype(BF16).reshape(G, 4, PC, P)
    xbig[:, 0:PC, 0] = pt[:, 0]
    xbig[:, 64 : 64 + PC, 0] = pt[:, 1]
    xbig[:, 0:PC, 1] = pt[:, 2]
    xbig[:, 64 : 64 + PC, 1] = pt[:, 3]
    dd = xT[:, PC:, :].astype(BF16).reshape(G, 4, DC, P)
    for j in range(4):
        xbig[:, 32 * j : 32 * j + DC, 2] = dd[:, j]
    xbig = (
        xbig.reshape(SG, R, 128, XW).transpose(0, 2, 1, 3).reshape(SG, 128, R * XW)
    )
    xbig = np.ascontiguousarray(xbig)

    # L0 weights, feature-major lhsT canvas ([in,out] = W^T), 2 nets/canvas-half
    w0T = W0[lo:hi].transpose(0, 2, 1).astype(BF16).reshape(G, 4, PC, H)
    w0p = np.zeros((G, 128, 128), dtype=BF16)
    for j in range(4):
        r = 64 * (j % 2)
        w0p[:, r : r + PC, 32 * j : 32 * j + 32] = w0T[:, j]
    w0p = (
        w0p.reshape(SG, R, 128, 128).transpose(0, 2, 1, 3).reshape(SG, 128, R * 128)
    )
    w0p = np.ascontiguousarray(w0p)

    # fold the (linear, non-output) feature layer into the direction layer:
    # Wfd = Wd_f @ Wf, bfd = Wd_f @ bf + bd
    Wd_f = Wd[lo:hi, :, :H]  # [n, 32(out), 32(feat-in)]
    Wfd = np.matmul(Wd_f, Wf[lo:hi])  # [n, 32(out), 32(h2-in)]
    bfd = np.einsum("nof,nf->no", Wd_f, bf[lo:hi]) + bd[lo:hi]

    # diag blocks: [SG, strip j, 32 feat, R, mat, 32 out]
    wdiag = np.zeros((G, 4, 32, 5, 32), dtype=BF16)
    w1T = W1[lo:hi].transpose(0, 2, 1).astype(BF16).reshape(G, 4, H, H)
    wfdT = Wfd.transpose(0, 2, 1).astype(BF16).reshape(G, 4, H, H)
    wddT = Wd[lo:hi, :, H:].transpose(0, 2, 1).astype(BF16).reshape(G, 4, DC, H)
    wrT = Wr[lo:hi].transpose(0, 2, 1).astype(BF16).reshape(G, 4, H, 3)
    waT = Wa[lo:hi].transpose(0, 2, 1).astype(BF16).reshape(G, 4, H, 1)
    wdiag[:, :, :, 0, :] = w1T
    wdiag[:, :, :, 1, :] = wfdT
    wdiag[:, :, :DC, 2, :] = wddT
    wdiag[:, :, :, 3, 0:3] = wrT
    wdiag[:, :, :, 4, 3:4] = waT
    wdiag = np.ascontiguousarray(
        wdiag.reshape(SG, R, 4, 32, 5, 32).transpose(0, 2, 3, 1, 4, 5)
    )  # -> [SG, j, 32, R, 5, 32]

    # biases: [128, G*4] with cols (b0, b1, bfd, bout) per group
    bias = np.zeros((G, 128, 4), dtype=np.float32)
    bias[:, :, 0] = b0[lo:hi].reshape(G, 128)
    bias[:, :, 1] = b1[lo:hi].reshape(G, 128)
    bias[:, :, 2] = bfd.reshape(G, 128)
    bo = np.zeros((G, 4, 32), dtype=np.float32)
    bo[:, :, 0:3] = br[lo:hi].reshape(G, 4, 3)
    bo[:, :, 3] = ba[lo:hi].reshape(G, 4)
    bias[:, :, 3] = bo.reshape(G, 128)
    bias = np.ascontiguousarray(bias.transpose(1, 0, 2).reshape(128, G * 4))

    return {
        "xin": xbig,
        "w0": w0p,
        "wdiag": wdiag,
        "bias": bias,
    }


def kernel(**inputs):
    from concourse.bass_utils import run_bass_kernel_spmd

    if "nc" not in _nc_cache:
        _nc_cache["nc"] = _build_nc()
    nc = _nc_cache["nc"]

    from concurrent.futures import ThreadPoolExecutor

    with ThreadPoolExecutor(max_workers=8) as ex:
        in_maps = list(ex.map(lambda c: _pack_core(c, **inputs), range(NCORES)))

    res = run_bass_kernel_spmd(nc, in_maps, core_ids=list(range(NCORES)))

    out = np.empty((N_NET, P, 4), dtype=np.float32)
    for c in range(NCORES):
        o = res.results[c]["out"]  # [SG, 4(net j), 4(chan), R, P]
        # net index within core = 4*(R*s + r) + j
        o = o.transpose(0, 3, 1, 4, 2)  # [SG, R, j, P, chan]
        out[c * NPC : (c + 1) * NPC] = o.reshape(NPC, P, 4)
    return out


# revision 6
# speedup vs baseline: 3.8619x; 1.6030x over previous
"""KiloNeRF Trainium2 kernel: 4096 tiny MLPs, 512 points each, 8 NeuronCores.

Sharding: expert-parallel along the network axis (512 nets/core). Host-side
numpy packs per-core inputs into feature-major, PE-friendly layouts; the
device kernel is a stream of full-array block-diagonal matmuls (4 nets per
128-partition tile), bf16 inputs with f32 PSUM accumulation.

v4:
- Feature layer folded into direction layer (7 matmuls / group).
- All transfers batched per supergroup of 8 groups, double-buffered.
- Weight canvases shipped DENSE (64B-run strip descriptors were 93% of DMA
  queue time at ~39 ns/descriptor); no on-device memset needed.
- Lout uses compact 16-column canvases; 4 groups share one PSUM bank so the
  output eviction runs once per 4 groups.
- DMA issue split across both HWDGE rings (Sync + Scalar sequencers).
"""

import sys

sys.path.insert(0, "/opt/trn_rl_repo")

import numpy as np
import ml_dtypes

N_NET = 4096
P = 512
PC = 63
DC = 27
H = 32
NCORES = 8
NPC = N_NET // NCORES  # nets per core = 512
NPG = 4  # nets per group (one 128-partition tile)
G = NPC // NPG  # groups per core = 128
R = 8  # groups per supergroup
SG = G // R  # supergroups = 16

BF16 = ml_dtypes.bfloat16

_nc_cache = {}


def _build_nc():
    import concourse.mybir as mybir
    import concourse.tile as tile
    from concourse import bacc

    nc = bacc.Bacc("TRN2")
    dt = mybir.dt
    AF = mybir.ActivationFunctionType
    ALU = mybir.AluOpType

    XW = 3 * P  # x cols per group: pos0 | pos1 | dir
    CW = 3 * 128  # dense canvas cols per group: w1 | wfd | wdd
    LW = 32  # compact lout canvas cols per group: wr(16) | wa(16)

    with tile.TileContext(nc) as tc:
        x_d = nc.dram_tensor("xin", [SG, 128, R * XW], dt.bfloat16, kind="ExternalInput")
        w0_d = nc.dram_tensor("w0", [SG, 128, R * 128], dt.bfloat16, kind="ExternalInput")
        cv_d = nc.dram_tensor("cv", [SG, 128, R * CW], dt.bfloat16, kind="ExternalInput")
        lo_d = nc.dram_tensor("lo", [SG, 128, R * LW], dt.bfloat16, kind="ExternalInput")
        biasg_d = nc.dram_tensor("biasg", [128, G * 4], dt.float32, kind="ExternalInput")
        boutb_d = nc.dram_tensor("boutb", [128, G // 4], dt.float32, kind="ExternalInput")
        out_d = nc.dram_tensor("out", [SG, 4, 16, 2, P], dt.float32, kind="ExternalOutput")

        with (
            tc.tile_pool(name="big", bufs=1) as bigp,
            tc.tile_pool(name="act", bufs=3) as actp,
            tc.tile_pool(name="ob", bufs=2) as obp,
            tc.tile_pool(name="ps0", bufs=2, space="PSUM") as ps0,
            tc.tile_pool(name="ps1", bufs=2, space="PSUM") as ps1,
            tc.tile_pool(name="psd", bufs=2, space="PSUM") as psd,
            tc.tile_pool(name="pso", bufs=2, space="PSUM") as pso,
        ):
            biasg = bigp.tile([128, G * 4], dt.float32, tag="biasg")
            boutb = bigp.tile([128, G // 4], dt.float32, tag="boutb")
            nc.sync.dma_start(out=biasg[:], in_=biasg_d[:])
            nc.scalar.dma_start(out=boutb[:], in_=boutb_d[:])

            xbig = [bigp.tile([128, R * XW], dt.bfloat16, tag=f"x{i}", name=f"x{i}") for i in range(2)]
            w0big = [bigp.tile([128, R * 128], dt.bfloat16, tag=f"w0{i}", name=f"w0{i}") for i in range(2)]
            cvbig = [bigp.tile([128, R * CW], dt.bfloat16, tag=f"cv{i}", name=f"cv{i}") for i in range(2)]
            lobig = [bigp.tile([128, R * LW], dt.bfloat16, tag=f"lo{i}", name=f"lo{i}") for i in range(2)]

            for s in range(SG):
                b = s % 2
                xb, w0b, cvb, lob = xbig[b], w0big[b], cvbig[b], lobig[b]
                # split issue across both HWDGE rings
                half = R * XW // 2
                nc.sync.dma_start(out=xb[:, 0:half], in_=x_d[s, :, 0:half])
                nc.scalar.dma_start(out=xb[:, half:], in_=x_d[s, :, half:])
                nc.sync.dma_start(out=w0b[:], in_=w0_d[s])
                nc.scalar.dma_start(out=cvb[:], in_=cv_d[s])
                nc.scalar.dma_start(out=lob[:], in_=lo_d[s])

                obstage = obp.tile([128, 2 * P], dt.float32, tag="obstage")
                for r in range(R):
                    g = R * s + r
                    h, q = r // 4, r % 4
                    pos0 = xb[:, r * XW : r * XW + P]
                    pos1 = xb[:, r * XW + P : r * XW + 2 * P]
                    dirt = xb[:, r * XW + 2 * P : r * XW + 3 * P]
                    w0 = w0b[:, r * 128 : (r + 1) * 128]
                    cv = cvb[:, r * CW : (r + 1) * CW]

                    def bia(i):
                        return biasg[:, 4 * g + i : 4 * g + i + 1]

                    # --- L0: h1 = relu(pos @ W0^T + b0) ---
                    p_l0 = ps0.tile([128, P], dt.float32, tag="l0")
                    nc.tensor.matmul(p_l0[0:64], lhsT=w0[:, 0:64], rhs=pos0, start=True, stop=True)
                    nc.tensor.matmul(p_l0[64:128], lhsT=w0[:, 64:128], rhs=pos1, start=True, stop=True)
                    h1 = actp.tile([128, P], dt.bfloat16, tag="h1")
                    nc.scalar.activation(h1[:], p_l0[:], AF.Relu, bias=bia(0), scale=1.0)

                    # --- L1: h2 = relu(h1 @ W1^T + b1) ---
                    p_l1 = ps1.tile([128, P], dt.float32, tag="l1")
                    nc.tensor.matmul(p_l1[:], lhsT=cv[:, 0:128], rhs=h1[:], start=True, stop=True)
                    h2 = actp.tile([128, P], dt.bfloat16, tag="h2")
                    nc.vector.tensor_scalar(h2[:], p_l1[:], bia(1), 0.0, op0=ALU.add, op1=ALU.max)

                    # --- Ld: h3 = relu(h2 @ Wfd^T + dir @ Wdd^T + bfd) ---
                    p_ld = psd.tile([128, P], dt.float32, tag="ld")
                    nc.tensor.matmul(p_ld[:], lhsT=cv[:, 128:256], rhs=h2[:], start=True, stop=False)
                    nc.tensor.matmul(p_ld[:], lhsT=cv[:, 256:384], rhs=dirt, start=False, stop=True)
                    h3 = actp.tile([128, P], dt.bfloat16, tag="h3")
                    nc.vector.tensor_scalar(h3[:], p_ld[:], bia(2), 0.0, op0=ALU.add, op1=ALU.max)

                    # --- Lout (compact, 4 groups per PSUM bank):
                    #     psum rows 32q+4j+k = net j, chan k (rgb,alpha) ---
                    if q == 0:
                        p_lo4 = pso.tile([128, P], dt.float32, tag="lo4")
                    nc.tensor.matmul(
                        p_lo4[32 * q : 32 * q + 16],
                        lhsT=lob[:, r * LW : r * LW + 16],
                        rhs=h3[:],
                        start=True,
                        stop=False,
                        tile_position=(0, 32 * q),
                    )
                    nc.tensor.matmul(
                        p_lo4[32 * q : 32 * q + 16],
                        lhsT=lob[:, r * LW + 16 : r * LW + 32],
                        rhs=h2[:],
                        start=False,
                        stop=True,
                        tile_position=(0, 32 * q),
                    )
                    if q == 3:
                        # one eviction per 4 groups; garbage rows are never read
                        nc.scalar.activation(
                            obstage[:, h * P : (h + 1) * P],
                            p_lo4[:],
                            AF.Identity,
                            bias=boutb[:, 2 * s + h : 2 * s + h + 1],
                            scale=1.0,
                        )

                for q in range(4):
                    eng = nc.sync if q % 2 == 0 else nc.scalar
                    eng.dma_start(
                        out=out_d[s, q],
                        in_=obstage[32 * q : 32 * q + 16].rearrange("p (h c) -> p h c", h=2),
                    )

    nc.compile()
    return nc


def _pack_core(c, x, W0, b0, W1, b1, Wa, ba, Wf, bf, Wd, bd, Wr, br):
    lo, hi = c * NPC, (c + 1) * NPC
    XW = 3 * P
    xT = np.ascontiguousarray(
        x[lo:hi].transpose(0, 2, 1)
    )  # [512, 90, 512] f32 feature-major

    # x big tiles: [SG, 128, R*(pos0|pos1|dir)]
    xbig = np.zeros((G, 128, 3, P), dtype=BF16)
    pt = xT[:, :PC, :].astype(BF16).reshape(G, 4, PC, P)
    xbig[:, 0:PC, 0] = pt[:, 0]
    xbig[:, 64 : 64 + PC, 0] = pt[:, 1]
    xbig[:, 0:PC, 1] = pt[:, 2]
    xbig[:, 64 : 64 + PC, 1] = pt[:, 3]
    dd = xT[:, PC:, :].astype(BF16).reshape(G, 4, DC, P)
    for j in range(4):
        xbig[:, 32 * j : 32 * j + DC, 2] = dd[:, j]
    xbig = np.ascontiguousarray(
        xbig.reshape(SG, R, 128, XW).transpose(0, 2, 1, 3).reshape(SG, 128, R * XW)
    )

    # L0 weights, feature-major lhsT canvas ([in,out] = W^T), 2 nets/canvas-half
    w0T = W0[lo:hi].transpose(0, 2, 1).astype(BF16).reshape(G, 4, PC, H)
    w0p = np.zeros((G, 128, 128), dtype=BF16)
    for j in range(4):
        r = 64 * (j % 2)
        w0p[:, r : r + PC, 32 * j : 32 * j + 32] = w0T[:, j]
    w0p = np.ascontiguousarray(
        w0p.reshape(SG, R, 128, 128).transpose(0, 2, 1, 3).reshape(SG, 128, R * 128)
    )

    # fold the (linear, non-output) feature layer into the direction layer:
    # Wfd = Wd_f @ Wf, bfd = Wd_f @ bf + bd
    Wd_f = Wd[lo:hi, :, :H]  # [n, 32(out), 32(feat-in)]
    Wfd = np.matmul(Wd_f, Wf[lo:hi])  # [n, 32(out), 32(h2-in)]
    bfd = np.einsum("nof,nf->no", Wd_f, bf[lo:hi]) + bd[lo:hi]

    # dense block-diagonal canvases [G, 128, 3 mats, 128]
    w1T = W1[lo:hi].transpose(0, 2, 1).astype(BF16).reshape(G, 4, H, H)
    wfdT = Wfd.transpose(0, 2, 1).astype(BF16).reshape(G, 4, H, H)
    wddT = Wd[lo:hi, :, H:].transpose(0, 2, 1).astype(BF16).reshape(G, 4, DC, H)
    cvd = np.zeros((G, 128, 3, 128), dtype=BF16)
    for j in range(4):
        sl = slice(32 * j, 32 * j + 32)
        cvd[:, sl, 0, sl] = w1T[:, j]
        cvd[:, sl, 1, sl] = wfdT[:, j]
        cvd[:, 32 * j : 32 * j + DC, 2, sl] = wddT[:, j]
    cvd = np.ascontiguousarray(
        cvd.reshape(SG, R, 128, 384).transpose(0, 2, 1, 3).reshape(SG, 128, R * 384)
    )

    # compact lout mini-canvases [G, 128, 2, 16]: wr cols 4j..4j+2, wa col 4j+3
    wrT = Wr[lo:hi].transpose(0, 2, 1).astype(BF16).reshape(G, 4, H, 3)
    waT = Wa[lo:hi].transpose(0, 2, 1).astype(BF16).reshape(G, 4, H, 1)
    lod = np.zeros((G, 128, 2, 16), dtype=BF16)
    for j in range(4):
        lod[:, 32 * j : 32 * j + 32, 0, 4 * j : 4 * j + 3] = wrT[:, j]
        lod[:, 32 * j : 32 * j + 32, 1, 4 * j + 3] = waT[:, j, :, 0]
    lod = np.ascontiguousarray(
        lod.reshape(SG, R, 128, 32).transpose(0, 2, 1, 3).reshape(SG, 128, R * 32)
    )

    # biases: per-group cols (b0, b1, bfd, -) and per-bank lout bias
    biasg = np.zeros((G, 128, 4), dtype=np.float32)
    biasg[:, :, 0] = b0[lo:hi].reshape(G, 128)
    biasg[:, :, 1] = b1[lo:hi].reshape(G, 128)
    biasg[:, :, 2] = bfd.reshape(G, 128)
    biasg = np.ascontiguousarray(biasg.transpose(1, 0, 2).reshape(128, G * 4))

    bo = np.zeros((G, 4, 4), dtype=np.float32)  # [group, net j, chan k]
    bo[:, :, 0:3] = br[lo:hi].reshape(G, 4, 3)
    bo[:, :, 3] = ba[lo:hi].reshape(G, 4)
    boutb = np.zeros((128, G // 4), dtype=np.float32)
    for q in range(4):
        # bank B holds groups 4B..4B+3; group 4B+q sits at rows 32q+4j+k
        boutb[32 * q : 32 * q + 16, :] = (
            bo[q::4].reshape(G // 4, 16).T
        )
    return {
        "xin": xbig,
        "w0": w0p,
        "cv": cvd,
        "lo": lod,
        "biasg": biasg,
        "boutb": boutb,
    }


def kernel(**inputs):
    from concourse.bass_utils import run_bass_kernel_spmd

    if "nc" not in _nc_cache:
        _nc_cache["nc"] = _build_nc()
    nc = _nc_cache["nc"]

    from concurrent.futures import ThreadPoolExecutor

    with ThreadPoolExecutor(max_workers=8) as ex:
        in_maps = list(ex.map(lambda c: _pack_core(c, **inputs), range(NCORES)))

    res = run_bass_kernel_spmd(nc, in_maps, core_ids=list(range(NCORES)))

    out = np.empty((N_NET, P, 4), dtype=np.float32)
    for c in range(NCORES):
        o = res.results[c]["out"]  # [SG, q, 16(4j+k), 2(h), P]
        o = o.reshape(SG, 4, 4, 4, 2, P)  # [s, q, j, k, h, c]
        o = o.transpose(0, 4, 1, 2, 5, 3)  # [s, h, q, j, c, k]
        out[c * NPC : (c + 1) * NPC] = o.reshape(NPC, P, 4)
    return out
